# revision 9
# baseline (speedup 1.0000x reference)
"""Trainium2 Bass kernel for nn_CrossAttention (B=2, S=64x64=4096, dim=256, 8 heads).

Sharding: 16 (batch, head) attention units across 8 cores -> 2 heads per core,
4 cores per batch. Projection weights are sliced per core on the host; the
small output-projection partial sums (4 per batch) are combined on the host.

Per-core device program (all cores run the same program, SPMD):
  inputs (host-pretransposed):
    qT  [256, 4096]  query[b]^T          sT  [256, 4096]  sim[b]^T
    wq/wk [128, 128]  two 128-row chunks of the per-core [256, 64] weight slice
    bq/bk [64, 1]     per-partition biases (wq/bq pre-scaled by dh^-0.5)
    wv  [128, 132]    chunks of [256, 66] = [Wv_h0 | 0 | Wv_h1 | 0] (aug cols)
    bv  [1, 66]       [bv_h0 | 1 | bv_h1 | 1]  (the 1s build the ones-column of
                      v_aug so the attention denominator falls out of the AV
                      matmul for free)
    wp  [64, 256]     out-projection rows for this core's 2 heads
  output:
    outT [256, 4096]  partial out-projection, transposed

  stage 1: qT_both/kT_both [64, S]  (rows 0-31 head0, 32-63 head1)
  stage 2: v_aug [S/128 tiles, 128, 66] in one sbuf tensor
  stage 3: per q-block of 512: for each of S/128 k-tiles:
             scoresT [128, 1024] (2 heads) on PE -> exp on ACT -> AV matmul
             accumulating [33, 512] per head (row 32 = denominator)
           then normalize via reciprocal + PE broadcast -> xT_both [64, S]
  stage 4: outT = wp^T @ xT_both
"""

import numpy as np

import concourse.bass as bass
import concourse.mybir as mybir
import concourse.tile as tile
from concourse import bacc, bass_utils

F32 = mybir.dt.float32
F16 = mybir.dt.float16
Exp = mybir.ActivationFunctionType.Exp

DIM = 256
NH = 8
DH = 32
B = 2
HGT = 64
WID = 64
S_FULL = HGT * WID  # 4096
N_CORES = 8
QB = 512  # q-block (free dim of scores matmuls / AV accumulation)
KT = 128  # k-tile (partition dim of scoresT tiles)


def build_bass(S=S_FULL):
    nqb = S // QB
    nkt = S // KT
    nc = bacc.Bacc("TRN2", target_bir_lowering=False, debug=False,
                   num_devices=N_CORES)

    qT_d = nc.dram_tensor("qT", [DIM, S], F16, kind="ExternalInput").ap()
    sT_d = nc.dram_tensor("sT", [DIM, S], F16, kind="ExternalInput").ap()
    wq_d = nc.dram_tensor("wq", [128, 128], F16, kind="ExternalInput").ap()
    wk_d = nc.dram_tensor("wk", [128, 128], F16, kind="ExternalInput").ap()
    bq_d = nc.dram_tensor("bq", [64, 1], F32, kind="ExternalInput").ap()
    bk_d = nc.dram_tensor("bk", [64, 1], F32, kind="ExternalInput").ap()
    wv_d = nc.dram_tensor("wv", [128, 132], F16, kind="ExternalInput").ap()
    bv_d = nc.dram_tensor("bv", [1, 66], F16, kind="ExternalInput").ap()
    wp_d = nc.dram_tensor("wp", [64, 256], F32, kind="ExternalInput").ap()
    outT_d = nc.dram_tensor("outT", [DIM, S], F32, kind="ExternalOutput").ap()

    with tile.TileContext(nc) as tc:
        with (
            tc.tile_pool(name="wpool", bufs=1) as wpool,
            tc.tile_pool(name="io", bufs=1) as io,
            tc.tile_pool(name="qk", bufs=1) as qk,
            tc.tile_pool(name="vx", bufs=1) as vx,
            tc.tile_pool(name="at", bufs=4) as atp,
            tc.tile_pool(name="sml", bufs=2) as sml,
            tc.tile_pool(name="ob", bufs=4) as obp,
        ):
            # --- constant / weight tiles ---
            wq_sb = wpool.tile([128, 128], F16, name="wq_sb", tag="wq")
            wk_sb = wpool.tile([128, 128], F16, name="wk_sb", tag="wk")
            wv_sb = wpool.tile([128, 132], F16, name="wv_sb", tag="wv")
            wp_sb = wpool.tile([64, 256], F32, name="wp_sb", tag="wp")
            bq_sb = wpool.tile([64, 1], F32, name="bq_sb", tag="bq")
            bk_sb = wpool.tile([64, 1], F32, name="bk_sb", tag="bk")
            bv_sb = wpool.tile([1, 66], F16, name="bv_sb", tag="bv")
            ones_row = wpool.tile([1, 128], F16, name="ones_row", tag="onesr")
            nc.sync.dma_start(wq_sb[:], wq_d)
            nc.sync.dma_start(wk_sb[:], wk_d)
            nc.sync.dma_start(wv_sb[:], wv_d)
            nc.sync.dma_start(wp_sb[:], wp_d)
            nc.sync.dma_start(bq_sb[:], bq_d)
            nc.sync.dma_start(bk_sb[:], bk_d)
            nc.sync.dma_start(bv_sb[:], bv_d)
            nc.vector.memset(ones_row[:], 1.0)

            # --- input activations, tiled [chunk][s-block] = [128, QB] ---
            qin = [[None] * nqb for _ in range(2)]
            sin = [[None] * nqb for _ in range(2)]
            for sb in range(nqb):
                for c in range(2):
                    t = io.tile([128, QB], F16, name=f"sin{c}_{sb}", tag="sin",
                                bufs=2 * nqb)
                    nc.sync.dma_start(
                        t[:], sT_d[c * 128:(c + 1) * 128, sb * QB:(sb + 1) * QB])
                    sin[c][sb] = t
                for c in range(2):
                    t = io.tile([128, QB], F16, name=f"qin{c}_{sb}", tag="qin",
                                bufs=2 * nqb)
                    nc.sync.dma_start(
                        t[:], qT_d[c * 128:(c + 1) * 128, sb * QB:(sb + 1) * QB])
                    qin[c][sb] = t

            qT = qk.tile([64, S], F16, name="qT_both", tag="qT")
            kT = qk.tile([64, S], F16, name="kT_both", tag="kT")
            v_sb = vx.tile([128, 66 * nkt], F16, name="v_sb", tag="v")
            xT = vx.tile([64, S], F32, name="xT_both", tag="xT")

            # --- projections + attention, interleaved so the ACT (exp)
            # pipeline starts as early as possible and PE work hides under it.
            with (
                tc.tile_pool(name="aux_ps", bufs=2,
                             space=bass.MemorySpace.PSUM) as aux_ps,
                tc.tile_pool(name="sc_ps", bufs=2,
                             space=bass.MemorySpace.PSUM) as sc_ps,
                tc.tile_pool(name="av_ps", bufs=2,
                             space=bass.MemorySpace.PSUM) as av_ps,
            ):
                def qkproj(w_sb, b_sb, srcin, dst, sb):
                    p = aux_ps.tile([64, QB], F32, name=f"p_{sb}", tag="aux")
                    nc.tensor.matmul(p[:], w_sb[:, 0:64], srcin[0][sb][:],
                                     start=True, stop=False)
                    nc.tensor.matmul(p[:], w_sb[:, 64:128], srcin[1][sb][:],
                                     start=False, stop=True)
                    nc.vector.tensor_scalar_add(
                        dst[:, sb * QB:(sb + 1) * QB], p[:], b_sb[:])

                def vproj(st):
                    sb, off = divmod(st * KT, QB)
                    pv = aux_ps.tile([128, 66], F32, name=f"pv_{st}", tag="aux")
                    nc.tensor.matmul(pv[:], sin[0][sb][:, off:off + KT],
                                     wv_sb[:, 0:66], start=True, stop=False)
                    nc.tensor.matmul(pv[:], sin[1][sb][:, off:off + KT],
                                     wv_sb[:, 66:132], start=False, stop=False)
                    nc.tensor.matmul(pv[:], ones_row[:, 0:KT], bv_sb[:],
                                     start=False, stop=True)
                    nc.vector.tensor_copy(v_sb[:, st * 66:(st + 1) * 66], pv[:])

                # minimal prologue: just enough for attention (qb0, kt0..3)
                qkproj(wk_sb, bk_sb, sin, kT, 0)
                qkproj(wq_sb, bq_sb, qin, qT, 0)
                vproj(0)
                vproj(1)

                def normalize(pav, pqb, phase):
                    """Normalize + out-projection for a finished q-block.
                    phase 0: reciprocals; 1: broadcast+muls; 2..9: out-proj
                    pieces (N=128 each); 10: copy-out + DMA. Spreading the
                    phases across the next q-block's kt steps keeps the PE
                    stream free of long serial stalls."""
                    pqs = slice(pqb * QB, (pqb + 1) * QB)
                    st_ = state[pqb]
                    if phase == 0:
                        r0 = sml.tile([1, QB], F32, name=f"r0_{pqb}", tag="r0")
                        r1 = sml.tile([1, QB], F32, name=f"r1_{pqb}", tag="r1")
                        nc.vector.reciprocal(r0[:], pav[32:33, :])
                        nc.vector.reciprocal(r1[:], pav[96:97, :])
                        st_["r"] = (r0, r1)
                    elif phase == 1:
                        r0, r1 = st_["r"]
                        bc0 = sml.tile([32, QB], F32, name=f"bc0_{pqb}", tag="bc0")
                        bc1 = sml.tile([32, QB], F32, name=f"bc1_{pqb}", tag="bc1")
                        nc.gpsimd.partition_broadcast(bc0[:, :], r0[:])
                        nc.gpsimd.partition_broadcast(bc1[:, :], r1[:])
                        nc.vector.tensor_mul(xT[0:32, pqs], pav[0:32, :], bc0[:, :])
                        nc.vector.tensor_mul(xT[32:64, pqs], pav[64:96, :], bc1[:, :])
                    elif phase <= 9:
                        i = phase - 2
                        ob, seg = divmod(i, 4)
                        if seg == 0:
                            st_[ob] = aux_ps.tile([128, QB], F32,
                                                  name=f"po_{ob}_{pqb}", tag="aux")
                        po = st_[ob]
                        nc.tensor.matmul(
                            po[:, seg * 128:(seg + 1) * 128],
                            wp_sb[:, ob * 128:(ob + 1) * 128],
                            xT[:, pqb * QB + seg * 128:pqb * QB + (seg + 1) * 128],
                            start=(seg == 0), stop=(seg == 3))
                    else:
                        for ob in range(2):
                            po = st_[ob]
                            osb = obp.tile([128, QB], F32,
                                           name=f"os_{ob}_{pqb}", tag="os")
                            nc.vector.tensor_copy(osb[:], po[:])
                            nc.sync.dma_start(
                                outT_d[ob * 128:(ob + 1) * 128, pqs], osb[:])

                state = [dict() for _ in range(nqb)]
                prev = None
                for qb in range(nqb):
                    qs = slice(qb * QB, (qb + 1) * QB)
                    av = av_ps.tile([128, QB], F32, name=f"av_{qb}", tag="av")
                    for kt in range(nkt):
                        if qb == 0:
                            # stream the remaining projections ahead of use
                            if kt % 4 == 0 and kt // 4 + 1 < nqb:
                                qkproj(wk_sb, bk_sb, sin, kT, kt // 4 + 1)
                            if kt % 4 == 1 and kt // 4 + 1 < nqb:
                                qkproj(wq_sb, bq_sb, qin, qT, kt // 4 + 1)
                            if kt + 2 < nkt:
                                vproj(kt + 2)
                        elif prev is not None and kt <= 20 and kt % 2 == 0:
                            normalize(prev[0], prev[1], kt // 2)
                        ks = slice(kt * KT, (kt + 1) * KT)
                        sc = sc_ps.tile([128, 2 * QB], F32,
                                        name=f"sc_{qb}_{kt}", tag="sc")
                        nc.tensor.matmul(sc[:, 0:QB], kT[0:32, ks],
                                         qT[0:32, qs], start=True, stop=True)
                        nc.tensor.matmul(sc[:, QB:2 * QB], kT[32:64, ks],
                                         qT[32:64, qs], start=True, stop=True)
                        at = atp.tile([128, 2 * QB], F16,
                                      name=f"at_{qb}_{kt}", tag="at")
                        nc.scalar.activation(at[:], sc[:], Exp)
                        nc.tensor.matmul(av[0:33, :],
                                         v_sb[:, kt * 66:kt * 66 + 33],
                                         at[:, 0:QB],
                                         start=(kt == 0), stop=(kt == nkt - 1),
                                         skip_group_check=True)
                        nc.tensor.matmul(av[64:97, :],
                                         v_sb[:, kt * 66 + 33:kt * 66 + 66],
                                         at[:, QB:2 * QB],
                                         start=(kt == 0), stop=(kt == nkt - 1),
                                         skip_group_check=True)
                    prev = (av, qb)
                # drain the deferred pipeline for the last q-block(s)
                if nqb == 1:
                    for ph in range(11):
                        normalize(prev[0], prev[1], ph)
                else:
                    for ph in range(11):
                        normalize(prev[0], prev[1], ph)

    nc.compile()
    return nc


def make_in_maps(query, sim, Wq, bq, Wkv, bkv, Wp, bp, S=S_FULL):
    query = np.asarray(query, dtype=np.float32)
    sim = np.asarray(sim, dtype=np.float32)
    Wq = np.asarray(Wq, dtype=np.float32)
    bq = np.asarray(bq, dtype=np.float32)
    Wkv = np.asarray(Wkv, dtype=np.float32)
    bkv = np.asarray(bkv, dtype=np.float32)
    Wp = np.asarray(Wp, dtype=np.float32)
    scale = np.float32(DH ** -0.5)
    in_maps = []
    for c in range(N_CORES):
        b = c // 4
        hh = (c % 4) * 2  # first of this core's two heads
        cq = slice(hh * DH, (hh + 2) * DH)
        qT = np.ascontiguousarray(query[b].reshape(S, DIM).T)
        sT = np.ascontiguousarray(sim[b].reshape(S, DIM).T)
        wq_c = Wq[:, cq] * scale
        wk_c = Wkv[:, cq]
        wv_c = Wkv[:, DIM + hh * DH:DIM + (hh + 2) * DH]
        wv_aug = np.zeros((DIM, 66), np.float32)
        wv_aug[:, 0:32] = wv_c[:, 0:32]
        wv_aug[:, 33:65] = wv_c[:, 32:64]
        bv_c = bkv[DIM + hh * DH:DIM + (hh + 2) * DH]
        bv_aug = np.zeros((1, 66), np.float32)
        bv_aug[0, 0:32] = bv_c[0:32]
        bv_aug[0, 32] = 1.0
        bv_aug[0, 33:65] = bv_c[32:64]
        bv_aug[0, 65] = 1.0
        in_maps.append({
            "qT": qT.astype(np.float16),
            "sT": sT.astype(np.float16),
            "wq": np.ascontiguousarray(
                np.concatenate([wq_c[:128], wq_c[128:]], axis=1)).astype(np.float16),
            "wk": np.ascontiguousarray(
                np.concatenate([wk_c[:128], wk_c[128:]], axis=1)).astype(np.float16),
            "bq": np.ascontiguousarray((bq[cq] * scale).reshape(64, 1)),
            "bk": np.ascontiguousarray(bkv[cq].reshape(64, 1)),
            "wv": np.ascontiguousarray(
                np.concatenate([wv_aug[:128], wv_aug[128:]], axis=1)).astype(np.float16),
            "bv": bv_aug.astype(np.float16),
            "wp": np.ascontiguousarray(Wp[cq, :]),
        })
    return in_maps


def gather_out(results, bp, S=S_FULL):
    bp = np.asarray(bp, dtype=np.float32)
    full = np.empty((B, S, DIM), np.float32)
    for b in range(B):
        acc = results[4 * b]["outT"].astype(np.float32)
        for c in range(4 * b + 1, 4 * b + 4):
            acc = acc + results[c]["outT"]
        full[b] = acc.T + bp[None, :]
    return full.reshape(B, S // WID, WID, DIM)


_NC_CACHE = {}


def _get_nc(S=S_FULL):
    if S not in _NC_CACHE:
        _NC_CACHE[S] = build_bass(S)
    return _NC_CACHE[S]


def run(inputs, trace=False, **kw):
    nc = _get_nc()
    in_maps = make_in_maps(**inputs)
    res = bass_utils.run_bass_kernel_spmd(
        nc, in_maps, core_ids=list(range(N_CORES)), trace=trace, **kw)
    return gather_out(res.results, inputs["bp"]), res


def kernel(**inputs):
    out, _ = run(inputs, trace=False)
    return out


# revision 10
# speedup vs baseline: 1.0089x; 1.0089x over previous
"""Trainium2 Bass kernel for nn_CrossAttention (B=2, S=64x64=4096, dim=256, 8 heads).

Sharding: 16 (batch, head) attention units across 8 cores -> 2 heads per core,
4 cores per batch. Projection weights are sliced per core on the host; the
small output-projection partial sums (4 per batch) are combined on the host.

Per-core device program (all cores run the same program, SPMD):
  inputs (host-pretransposed):
    qT  [256, 4096]  query[b]^T          sT  [256, 4096]  sim[b]^T
    wq/wk [128, 128]  two 128-row chunks of the per-core [256, 64] weight slice
    bq/bk [64, 1]     per-partition biases (wq/bq pre-scaled by dh^-0.5)
    wv  [128, 132]    chunks of [256, 66] = [Wv_h0 | 0 | Wv_h1 | 0] (aug cols)
    bv  [1, 66]       [bv_h0 | 1 | bv_h1 | 1]  (the 1s build the ones-column of
                      v_aug so the attention denominator falls out of the AV
                      matmul for free)
    wp  [64, 256]     out-projection rows for this core's 2 heads
  output:
    outT [256, 4096]  partial out-projection, transposed

  stage 1: qT_both/kT_both [64, S]  (rows 0-31 head0, 32-63 head1)
  stage 2: v_aug [S/128 tiles, 128, 66] in one sbuf tensor
  stage 3: per q-block of 512: for each of S/128 k-tiles:
             scoresT [128, 1024] (2 heads) on PE -> exp on ACT -> AV matmul
             accumulating [33, 512] per head (row 32 = denominator)
           then normalize via reciprocal + PE broadcast -> xT_both [64, S]
  stage 4: outT = wp^T @ xT_both
"""

import numpy as np

import concourse.bass as bass
import concourse.mybir as mybir
import concourse.tile as tile
from concourse import bacc, bass_utils

F32 = mybir.dt.float32
F16 = mybir.dt.float16
Exp = mybir.ActivationFunctionType.Exp

DIM = 256
NH = 8
DH = 32
B = 2
HGT = 64
WID = 64
S_FULL = HGT * WID  # 4096
N_CORES = 8
QB = 512  # q-block (free dim of scores matmuls / AV accumulation)
KT = 128  # k-tile (partition dim of scoresT tiles)


def build_bass(S=S_FULL):
    nqb = S // QB
    nkt = S // KT
    nc = bacc.Bacc("TRN2", target_bir_lowering=False, debug=False,
                   num_devices=N_CORES)

    qT_d = nc.dram_tensor("qT", [DIM, S], F16, kind="ExternalInput").ap()
    sT_d = nc.dram_tensor("sT", [DIM, S], F16, kind="ExternalInput").ap()
    wq_d = nc.dram_tensor("wq", [128, 128], F16, kind="ExternalInput").ap()
    wk_d = nc.dram_tensor("wk", [128, 128], F16, kind="ExternalInput").ap()
    bq_d = nc.dram_tensor("bq", [64, 1], F32, kind="ExternalInput").ap()
    bk_d = nc.dram_tensor("bk", [64, 1], F32, kind="ExternalInput").ap()
    wv_d = nc.dram_tensor("wv", [128, 132], F16, kind="ExternalInput").ap()
    bv_d = nc.dram_tensor("bv", [1, 66], F16, kind="ExternalInput").ap()
    wp_d = nc.dram_tensor("wp", [64, 256], F32, kind="ExternalInput").ap()
    outT_d = nc.dram_tensor("outT", [DIM, S], F32, kind="ExternalOutput").ap()

    with tile.TileContext(nc) as tc:
        with (
            tc.tile_pool(name="wpool", bufs=1) as wpool,
            tc.tile_pool(name="io", bufs=1) as io,
            tc.tile_pool(name="qk", bufs=1) as qk,
            tc.tile_pool(name="vx", bufs=1) as vx,
            tc.tile_pool(name="at", bufs=4) as atp,
            tc.tile_pool(name="sml", bufs=2) as sml,
            tc.tile_pool(name="ob", bufs=4) as obp,
        ):
            # --- constant / weight tiles ---
            wq_sb = wpool.tile([128, 128], F16, name="wq_sb", tag="wq")
            wk_sb = wpool.tile([128, 128], F16, name="wk_sb", tag="wk")
            wv_sb = wpool.tile([128, 132], F16, name="wv_sb", tag="wv")
            wp_sb = wpool.tile([64, 256], F32, name="wp_sb", tag="wp")
            bq_sb = wpool.tile([64, 1], F32, name="bq_sb", tag="bq")
            bk_sb = wpool.tile([64, 1], F32, name="bk_sb", tag="bk")
            bv_sb = wpool.tile([1, 66], F16, name="bv_sb", tag="bv")
            ones_row = wpool.tile([1, 128], F16, name="ones_row", tag="onesr")
            nc.sync.dma_start(wq_sb[:], wq_d)
            nc.sync.dma_start(wk_sb[:], wk_d)
            nc.sync.dma_start(wv_sb[:], wv_d)
            nc.sync.dma_start(wp_sb[:], wp_d)
            nc.sync.dma_start(bq_sb[:], bq_d)
            nc.sync.dma_start(bk_sb[:], bk_d)
            nc.sync.dma_start(bv_sb[:], bv_d)
            nc.vector.memset(ones_row[:], 1.0)

            # --- input activations, tiled [chunk][s-block] = [128, QB] ---
            qin = [[None] * nqb for _ in range(2)]
            sin = [[None] * nqb for _ in range(2)]
            for sb in range(nqb):
                for c in range(2):
                    t = io.tile([128, QB], F16, name=f"sin{c}_{sb}", tag="sin",
                                bufs=2 * nqb)
                    nc.sync.dma_start(
                        t[:], sT_d[c * 128:(c + 1) * 128, sb * QB:(sb + 1) * QB])
                    sin[c][sb] = t
                for c in range(2):
                    t = io.tile([128, QB], F16, name=f"qin{c}_{sb}", tag="qin",
                                bufs=2 * nqb)
                    nc.sync.dma_start(
                        t[:], qT_d[c * 128:(c + 1) * 128, sb * QB:(sb + 1) * QB])
                    qin[c][sb] = t

            qT = qk.tile([64, S], F16, name="qT_both", tag="qT")
            kT = qk.tile([64, S], F16, name="kT_both", tag="kT")
            v_sb = vx.tile([128, 66 * nkt], F16, name="v_sb", tag="v")
            xT = vx.tile([64, S], F32, name="xT_both", tag="xT")

            # --- projections + attention, interleaved so the ACT (exp)
            # pipeline starts as early as possible and PE work hides under it.
            with (
                tc.tile_pool(name="aux_ps", bufs=2,
                             space=bass.MemorySpace.PSUM) as aux_ps,
                tc.tile_pool(name="sc_ps", bufs=2,
                             space=bass.MemorySpace.PSUM) as sc_ps,
                tc.tile_pool(name="av_ps", bufs=2,
                             space=bass.MemorySpace.PSUM) as av_ps,
            ):
                def qkproj(w_sb, b_sb, srcin, dst, sb):
                    p = aux_ps.tile([64, QB], F32, name=f"p_{sb}", tag="aux")
                    nc.tensor.matmul(p[:], w_sb[:, 0:64], srcin[0][sb][:],
                                     start=True, stop=False)
                    nc.tensor.matmul(p[:], w_sb[:, 64:128], srcin[1][sb][:],
                                     start=False, stop=True)
                    nc.vector.tensor_scalar_add(
                        dst[:, sb * QB:(sb + 1) * QB], p[:], b_sb[:])

                def vproj(st):
                    sb, off = divmod(st * KT, QB)
                    pv = aux_ps.tile([128, 66], F32, name=f"pv_{st}", tag="aux")
                    nc.tensor.matmul(pv[:], sin[0][sb][:, off:off + KT],
                                     wv_sb[:, 0:66], start=True, stop=False)
                    nc.tensor.matmul(pv[:], sin[1][sb][:, off:off + KT],
                                     wv_sb[:, 66:132], start=False, stop=False)
                    nc.tensor.matmul(pv[:], ones_row[:, 0:KT], bv_sb[:],
                                     start=False, stop=True)
                    nc.vector.tensor_copy(v_sb[:, st * 66:(st + 1) * 66], pv[:])

                # minimal prologue: just enough for attention (qb0, kt0..3)
                qkproj(wk_sb, bk_sb, sin, kT, 0)
                qkproj(wq_sb, bq_sb, qin, qT, 0)
                vproj(0)
                vproj(1)

                def normalize(pav, pqb, phase):
                    """Normalize + out-projection for a finished q-block.
                    phase 0: reciprocals; 1: broadcast+muls; 2..9: out-proj
                    pieces (N=128 each); 10: copy-out + DMA. Spreading the
                    phases across the next q-block's kt steps keeps the PE
                    stream free of long serial stalls."""
                    pqs = slice(pqb * QB, (pqb + 1) * QB)
                    st_ = state[pqb]
                    if phase == 0:
                        r0 = sml.tile([1, QB], F32, name=f"r0_{pqb}", tag="r0")
                        r1 = sml.tile([1, QB], F32, name=f"r1_{pqb}", tag="r1")
                        nc.vector.reciprocal(r0[:], pav[32:33, :])
                        nc.vector.reciprocal(r1[:], pav[96:97, :])
                        st_["r"] = (r0, r1)
                    elif phase == 1:
                        r0, r1 = st_["r"]
                        bc0 = sml.tile([32, QB], F32, name=f"bc0_{pqb}", tag="bc0")
                        bc1 = sml.tile([32, QB], F32, name=f"bc1_{pqb}", tag="bc1")
                        nc.gpsimd.partition_broadcast(bc0[:, :], r0[:])
                        nc.gpsimd.partition_broadcast(bc1[:, :], r1[:])
                        nc.vector.tensor_mul(xT[0:32, pqs], pav[0:32, :], bc0[:, :])
                        nc.vector.tensor_mul(xT[32:64, pqs], pav[64:96, :], bc1[:, :])
                    elif phase <= 9:
                        i = phase - 2
                        ob, seg = divmod(i, 4)
                        if seg == 0:
                            st_[ob] = aux_ps.tile([128, QB], F32,
                                                  name=f"po_{ob}_{pqb}", tag="aux")
                        po = st_[ob]
                        nc.tensor.matmul(
                            po[:, seg * 128:(seg + 1) * 128],
                            wp_sb[:, ob * 128:(ob + 1) * 128],
                            xT[:, pqb * QB + seg * 128:pqb * QB + (seg + 1) * 128],
                            start=(seg == 0), stop=(seg == 3))
                    else:
                        for ob in range(2):
                            po = st_[ob]
                            osb = obp.tile([128, QB], F32,
                                           name=f"os_{ob}_{pqb}", tag="os")
                            nc.vector.tensor_copy(osb[:], po[:])
                            nc.sync.dma_start(
                                outT_d[ob * 128:(ob + 1) * 128, pqs], osb[:])

                state = [dict() for _ in range(nqb)]
                prev = None
                for qb in range(nqb):
                    qs = slice(qb * QB, (qb + 1) * QB)
                    av = av_ps.tile([128, QB], F32, name=f"av_{qb}", tag="av")
                    for kt in range(nkt):
                        if qb == 0:
                            # stream the remaining projections ahead of use
                            if kt % 4 == 0 and kt // 4 + 1 < nqb:
                                qkproj(wk_sb, bk_sb, sin, kT, kt // 4 + 1)
                            if kt % 4 == 1 and kt // 4 + 1 < nqb:
                                qkproj(wq_sb, bq_sb, qin, qT, kt // 4 + 1)
                            if kt + 2 < nkt:
                                vproj(kt + 2)
                        elif prev is not None:
                            # phase schedule: recip at kt0, bcast+mul at kt2,
                            # out-proj pieces kt12..26, copy-out kt28 -- late
                            # enough that the PE stream never waits on the
                            # reciprocal chain.
                            if kt == 0:
                                normalize(prev[0], prev[1], 0)
                            elif kt == 2:
                                normalize(prev[0], prev[1], 1)
                            elif 12 <= kt <= 26 and kt % 2 == 0:
                                normalize(prev[0], prev[1], 2 + (kt - 12) // 2)
                            elif kt == 28:
                                normalize(prev[0], prev[1], 10)
                        ks = slice(kt * KT, (kt + 1) * KT)
                        sc = sc_ps.tile([128, 2 * QB], F32,
                                        name=f"sc_{qb}_{kt}", tag="sc")
                        nc.tensor.matmul(sc[:, 0:QB], kT[0:32, ks],
                                         qT[0:32, qs], start=True, stop=True)
                        nc.tensor.matmul(sc[:, QB:2 * QB], kT[32:64, ks],
                                         qT[32:64, qs], start=True, stop=True)
                        at = atp.tile([128, 2 * QB], F16,
                                      name=f"at_{qb}_{kt}", tag="at")
                        nc.scalar.activation(at[:], sc[:], Exp)
                        nc.tensor.matmul(av[0:33, :],
                                         v_sb[:, kt * 66:kt * 66 + 33],
                                         at[:, 0:QB],
                                         start=(kt == 0), stop=(kt == nkt - 1),
                                         skip_group_check=True)
                        nc.tensor.matmul(av[64:97, :],
                                         v_sb[:, kt * 66 + 33:kt * 66 + 66],
                                         at[:, QB:2 * QB],
                                         start=(kt == 0), stop=(kt == nkt - 1),
                                         skip_group_check=True)
                    prev = (av, qb)
                # drain the deferred pipeline for the last q-block(s)
                if nqb == 1:
                    for ph in range(11):
                        normalize(prev[0], prev[1], ph)
                else:
                    for ph in range(11):
                        normalize(prev[0], prev[1], ph)

    nc.compile()
    return nc


def make_in_maps(query, sim, Wq, bq, Wkv, bkv, Wp, bp, S=S_FULL):
    query = np.asarray(query, dtype=np.float32)
    sim = np.asarray(sim, dtype=np.float32)
    Wq = np.asarray(Wq, dtype=np.float32)
    bq = np.asarray(bq, dtype=np.float32)
    Wkv = np.asarray(Wkv, dtype=np.float32)
    bkv = np.asarray(bkv, dtype=np.float32)
    Wp = np.asarray(Wp, dtype=np.float32)
    scale = np.float32(DH ** -0.5)
    in_maps = []
    for c in range(N_CORES):
        b = c // 4
        hh = (c % 4) * 2  # first of this core's two heads
        cq = slice(hh * DH, (hh + 2) * DH)
        qT = np.ascontiguousarray(query[b].reshape(S, DIM).T)
        sT = np.ascontiguousarray(sim[b].reshape(S, DIM).T)
        wq_c = Wq[:, cq] * scale
        wk_c = Wkv[:, cq]
        wv_c = Wkv[:, DIM + hh * DH:DIM + (hh + 2) * DH]
        wv_aug = np.zeros((DIM, 66), np.float32)
        wv_aug[:, 0:32] = wv_c[:, 0:32]
        wv_aug[:, 33:65] = wv_c[:, 32:64]
        bv_c = bkv[DIM + hh * DH:DIM + (hh + 2) * DH]
        bv_aug = np.zeros((1, 66), np.float32)
        bv_aug[0, 0:32] = bv_c[0:32]
        bv_aug[0, 32] = 1.0
        bv_aug[0, 33:65] = bv_c[32:64]
        bv_aug[0, 65] = 1.0
        in_maps.append({
            "qT": qT.astype(np.float16),
            "sT": sT.astype(np.float16),
            "wq": np.ascontiguousarray(
                np.concatenate([wq_c[:128], wq_c[128:]], axis=1)).astype(np.float16),
            "wk": np.ascontiguousarray(
                np.concatenate([wk_c[:128], wk_c[128:]], axis=1)).astype(np.float16),
            "bq": np.ascontiguousarray((bq[cq] * scale).reshape(64, 1)),
            "bk": np.ascontiguousarray(bkv[cq].reshape(64, 1)),
            "wv": np.ascontiguousarray(
                np.concatenate([wv_aug[:128], wv_aug[128:]], axis=1)).astype(np.float16),
            "bv": bv_aug.astype(np.float16),
            "wp": np.ascontiguousarray(Wp[cq, :]),
        })
    return in_maps


def gather_out(results, bp, S=S_FULL):
    bp = np.asarray(bp, dtype=np.float32)
    full = np.empty((B, S, DIM), np.float32)
    for b in range(B):
        acc = results[4 * b]["outT"].astype(np.float32)
        for c in range(4 * b + 1, 4 * b + 4):
            acc = acc + results[c]["outT"]
        full[b] = acc.T + bp[None, :]
    return full.reshape(B, S // WID, WID, DIM)


_NC_CACHE = {}


def _get_nc(S=S_FULL):
    if S not in _NC_CACHE:
        _NC_CACHE[S] = build_bass(S)
    return _NC_CACHE[S]


def run(inputs, trace=False, **kw):
    nc = _get_nc()
    in_maps = make_in_maps(**inputs)
    res = bass_utils.run_bass_kernel_spmd(
        nc, in_maps, core_ids=list(range(N_CORES)), trace=trace, **kw)
    return gather_out(res.results, inputs["bp"]), res


def kernel(**inputs):
    out, _ = run(inputs, trace=False)
    return out
